# revision 1
# baseline (speedup 1.0000x reference)
"""Trainium2 Bass kernel for a 2-layer GRU (B=64, T=2048, I=16, H=256) + MLP regressor.

Strategy (v3):
  - Data parallel: batch 64 sharded as 8 sequences per NeuronCore.
  - Each core runs BOTH GRU layers, software-pipelined with a D-step skew so the
    two layers' serial gate chains hide under each other's TensorE work.
  - All recurrent matmul operands bf16 (stationary weights get FWL: 2x faster
    LDWEIGHTS; carried h state is bf16). Regressor path kept fp32.
  - Layout: gates-on-partitions. r,z-gate input projections are precomputed
    into PSUM chunk accumulators; the per-step recurrent matmuls accumulate
    ONTO them (start=False), so sigmoid reads PSUM directly - no DVE add.
  - Gate chain per step (short critical path):
      rz = sigmoid(psum_rz); tt = r*hn; t2 = tt + xn; n = tanh(t2);
      tmp = (z-1)*n; h = zh - tmp.   zh = z*h_prev runs on GPSIMD off-chain.
  - n-gate input projections go to a small SBUF ring (one strided ACT copy
    per chunk); regressor fused every CR steps (fp32, h2 cast on GPSIMD).
"""

import os
import sys

import numpy as np

if "/opt/trn_rl_repo" not in sys.path:
    sys.path.insert(0, "/opt/trn_rl_repo")

import concourse.bacc as bacc
import concourse.mybir as mybir
import concourse.tile as tile
from concourse.bass import ds, ts
from concourse.bass_utils import run_bass_kernel_spmd

# Problem constants (hardcoded per harness contract)
B_TOTAL = 64
N_CORES = 8
Bc = B_TOTAL // N_CORES  # 8 sequences per core
T = 2048
I_DIM = 16
H = 256
G = 3 * H  # 768 gate rows
C = 16  # gate-chunk size (PSUM rz accumulators, xn ring refill)
CR = 32  # regressor chunk size
S = 64  # ring size in steps
D = 24  # layer-1 skew (steps)

F32 = mybir.dt.float32
BF16 = mybir.dt.bfloat16

AF = mybir.ActivationFunctionType
ALU = mybir.AluOpType


def build_program(dt_compute=BF16, repeat=1):
    """Build + compile the SPMD program (identical on all 8 cores)."""
    DT = dt_compute
    XDT = F32 if X_F32 else DT
    nc = bacc.Bacc("TRN2", target_bir_lowering=False, debug=False,
                   num_devices=N_CORES)

    # ---- DRAM I/O ----
    xT_h = nc.dram_tensor("xT", [I_DIM + 1, T * Bc], XDT, kind="ExternalInput")
    wh0_h = nc.dram_tensor("wh0T", [H, G], DT, kind="ExternalInput")
    wih0_h = nc.dram_tensor("wih0T", [I_DIM + 1, G], XDT, kind="ExternalInput")
    wh1_h = nc.dram_tensor("wh1T", [H, G], DT, kind="ExternalInput")
    wih1_h = nc.dram_tensor("wih1T", [H, G], DT, kind="ExternalInput")
    w1_h = nc.dram_tensor("w1T", [H, H], F32, kind="ExternalInput")
    b1_h = nc.dram_tensor("b1c", [128, 2], F32, kind="ExternalInput")
    w2_h = nc.dram_tensor("w2c", [128, 2], F32, kind="ExternalInput")
    b2_h = nc.dram_tensor("b2c", [1, 1], F32, kind="ExternalInput")
    out_h = nc.dram_tensor("out", [T // CR, CR * Bc], F32, kind="ExternalOutput")
    dbg = os.environ.get("KDBG", "0") == "1"
    dbg_handles = {}
    if dbg:
        for nm in ("dbg_h1", "dbg_h2", "dbg_x0", "dbg_x1", "dbg_ps"):
            dbg_handles[nm] = nc.dram_tensor(nm, [128, 64 * 16], F32,
                                             kind="ExternalOutput")

    NB = Bc  # batch per core
    W = NB * 2  # 16: one h-state slot width (2 k-chunks x 8)
    CB = C * NB  # 128: columns per (m-chunk, gate-chunk)

    with tile.TileContext(nc) as tc:
        with (
            tc.tile_pool(name="cst", bufs=1) as cst,
            tc.tile_pool(name="work", bufs=3) as work,
            tc.tile_pool(name="prz", bufs=2, space="PSUM") as prz,
            tc.tile_pool(name="pgg", bufs=1, space="PSUM") as pgg,
            tc.tile_pool(name="pbig", bufs=2, space="PSUM") as pbig,
        ):
            # ---- persistent SBUF ----
            xT = cst.tile([I_DIM + 1, T * NB], XDT, tag="xT")
            wh0 = cst.tile([128, 12 * 128], DT, tag="wh0")
            wh1 = cst.tile([128, 12 * 128], DT, tag="wh1")
            wih0 = cst.tile([I_DIM + 1, G], XDT, tag="wih0")
            wih1 = cst.tile([128, 2 * G], DT, tag="wih1")
            w1 = cst.tile([128, 4 * 128], F32, tag="w1")
            w2 = cst.tile([128, 2], F32, tag="w2")
            b1 = cst.tile([128, 2], F32, tag="b1")
            b2 = cst.tile([1, 1], F32, tag="b2")
            xn0 = cst.tile([128, S * 2 * NB], F32, tag="xn0")  # n-gate ring
            xn1 = cst.tile([128, S * 2 * NB], F32, tag="xn1")
            h1h = cst.tile([128, S * W], DT, tag="h1h")  # h1.T history ring
            h2h = cst.tile([128, S * W], DT, tag="h2h")
            # persistent per-step n-gate psum: (layer, t%2) -> 16-col region
            png = pgg.tile([128, 4 * W], F32, tag="gg")

            # ---- load constants ----
            nc.sync.dma_start(xT[:], xT_h[:, :])
            for m in range(6):
                for k in range(2):
                    i = m * 2 + k
                    nc.sync.dma_start(wh0[:, ts(i, 128)],
                                      wh0_h[ds(k * 128, 128), ds(m * 128, 128)])
                    nc.sync.dma_start(wh1[:, ts(i, 128)],
                                      wh1_h[ds(k * 128, 128), ds(m * 128, 128)])
            nc.sync.dma_start(wih0[:], wih0_h[:, :])
            for k in range(2):
                nc.sync.dma_start(wih1[:, ts(k, G)], wih1_h[ds(k * 128, 128), :])
            for mm in range(2):
                for k in range(2):
                    nc.sync.dma_start(w1[:, ts(mm * 2 + k, 128)],
                                      w1_h[ds(k * 128, 128), ds(mm * 128, 128)])
            nc.sync.dma_start(w2[:], w2_h[:, :])
            nc.sync.dma_start(b1[:], b1_h[:, :])
            nc.sync.dma_start(b2[:], b2_h[:, :])

            # live rz-psum chunk accumulators, per layer, by chunk parity
            psA = {0: [None, None], 1: [None, None]}

            aux_q = []
            copv = os.environ.get("KCOPV", "0") == "1"

            def emit_xg0_chunk(c):
                """Layer-0 input projections for steps [c*C, (c+1)*C)."""
                xchunk = xT[:, ds(c * CB, CB)]
                state = {}

                def u_rz(m0):
                    def f():
                        if m0 == 0:
                            state["t_ps"] = prz.tile([128, 4 * CB], F32,
                                                     tag="rz0", name="tps0")
                            psA[0][c % 2] = state["t_ps"]
                        for m in (m0, m0 + 1):
                            nc.tensor.matmul(state["t_ps"][:, ds(m * CB, CB)],
                                             wih0[:, ts(m, 128)], xchunk,
                                             start=(m == 0), stop=(m == 3))
                    return f

                def u_n():
                    pn = pbig.tile([128, 2 * CB], F32, tag="big")
                    state["pn"] = pn
                    for i, m in enumerate((4, 5)):
                        nc.tensor.matmul(pn[:, ds(i * CB, CB)],
                                         wih0[:, ts(m, 128)], xchunk,
                                         start=True, stop=True)

                def u_copy():
                    pn = state["pn"]
                    dst = xn0[:, ds((c * C % S) * 2 * NB, C * 2 * NB)]
                    dst = dst.rearrange("p (s m b) -> p s m b", m=2, b=NB)
                    srcv = pn[:].rearrange("p (m s b) -> p s m b", m=2, b=NB)
                    if copv:
                        nc.vector.tensor_copy(dst, srcv)
                    else:
                        nc.scalar.activation(dst, srcv, AF.Copy)

                aux_q.extend([u_rz(0), u_rz(2), u_n, u_copy])

            def emit_xg1_chunk(c):
                """Layer-1 input projections for steps [c*C, (c+1)*C)."""
                base_step = (c * C) % S
                seg = h1h[:, ds(base_step * W, C * W)]
                seg = seg.rearrange("p (s k b) -> p k s b", k=2, b=NB)
                state = {}

                def u_rz(m0):
                    def f():
                        if m0 == 0:
                            state["t_ps"] = prz.tile([128, 4 * CB], F32,
                                                     tag="rz1", name="tps1")
                            psA[1][c % 2] = state["t_ps"]
                        for m in (m0, m0 + 1):
                            for k in range(2):
                                nc.tensor.matmul(
                                    state["t_ps"][:, ds(m * CB, CB)],
                                    wih1[:, ds(k * G + m * 128, 128)],
                                    seg[:, k],
                                    start=(m == 0 and k == 0),
                                    stop=(m == 3 and k == 1))
                    return f

                def u_n(i, m):
                    def f():
                        if i == 0:
                            state["pn"] = pbig.tile([128, 2 * CB], F32,
                                                    tag="big", name="pn1")
                        for k in range(2):
                            nc.tensor.matmul(state["pn"][:, ds(i * CB, CB)],
                                             wih1[:, ds(k * G + m * 128, 128)],
                                             seg[:, k],
                                             start=(k == 0), stop=(k == 1))
                    return f

                def u_copy():
                    pn = state["pn"]
                    dst = xn1[:, ds(base_step * 2 * NB, C * 2 * NB)]
                    dst = dst.rearrange("p (s m b) -> p s m b", m=2, b=NB)
                    srcv = pn[:].rearrange("p (m s b) -> p s m b", m=2, b=NB)
                    if copv:
                        nc.vector.tensor_copy(dst, srcv)
                    else:
                        nc.scalar.activation(dst, srcv, AF.Copy)

                aux_q.extend([u_rz(0), u_rz(2), u_n(0, 4), u_n(1, 5), u_copy])

            def bias_relu(out, ps, bcol):
                if copv:
                    nc.vector.tensor_scalar(out, ps, bcol, 0.0,
                                            ALU.add, ALU.max)
                else:
                    nc.scalar.activation(out, ps, AF.Relu, bias=bcol)

            def emit_regressor_chunk(rc):
                """relu(h2@W1.T+b1) @ W2.T + b2 -> relu -> out, for steps
                [rc*CR, (rc+1)*CR) of layer 1. fp32 path."""
                base_step = (rc * CR) % S
                state = {}

                def u_seg():
                    segf = work.tile([128, CR * W], F32, tag="segf",
                                     name="segf")
                    nc.gpsimd.tensor_copy(segf[:],
                                          h2h[:, ds(base_step * W, CR * W)])
                    state["seg"] = segf[:].rearrange("p (s k b) -> p k s b",
                                                     k=2, b=NB)
                    state["rT"] = work.tile([128, 2 * CR * NB], F32,
                                            tag="rT", name="rTt")

                def u_mm(mm, k):
                    def f():
                        if k == 0:
                            state[f"ps{mm}"] = pbig.tile([128, CR * NB], F32,
                                                         tag="big",
                                                         name="psr")
                        nc.tensor.matmul(state[f"ps{mm}"][:],
                                         w1[:, ts(mm * 2 + k, 128)],
                                         state["seg"][:, k],
                                         start=(k == 0), stop=(k == 1))
                        if k == 1:
                            bias_relu(state["rT"][:, ts(mm, CR * NB)],
                                      state[f"ps{mm}"][:], b1[:, ds(mm, 1)])
                    return f

                def u_out():
                    pot = pbig.tile([128, CR * NB], F32, tag="big",
                                    name="pot")
                    po = pot[0:1, :]
                    for k in range(2):
                        nc.tensor.matmul(po, w2[:, ds(k, 1)],
                                         state["rT"][:, ts(k, CR * NB)],
                                         start=(k == 0), stop=(k == 1))
                    oT = work.tile([1, CR * NB], F32, tag="oT", name="oT")
                    bias_relu(oT[:], po, b2[:, ds(0, 1)])
                    nc.sync.dma_start(out_h[ds(rc, 1), :], oT[:])

                aux_q.extend([u_seg, u_mm(0, 0), u_mm(0, 1), u_mm(1, 0),
                              u_mm(1, 1), u_out])

            mm_only = os.environ.get("KMMONLY", "0") == "1"
            no_mm = os.environ.get("KNOMM", "0") == "1"
            sig_split = os.environ.get("KSIGSPLIT", "0") == "1"

            def step_gen(layer, t):
                """Generator emitting one GRU step; yields between chain ops so
                the two layers' chains can be interleaved per-engine."""
                wh = wh0 if layer == 0 else wh1
                hist = h1h if layer == 0 else h2h
                xn = xn0 if layer == 0 else xn1
                t_ps = psA[layer][(t // C) % 2]
                tc_off = (t % C) * NB
                prev = (t - 1) % S
                cur = t % S
                nbase = (layer * 2 + t % 2) * W

                def mm_rz(ms):
                    for m in ms:
                        for k in range(2):
                            nc.tensor.matmul(
                                t_ps[:, ds(m * CB + tc_off, NB)],
                                wh[:, ts(m * 2 + k, 128)],
                                hist[:, ds(prev * W + k * NB, NB)],
                                start=False, stop=(k == 1),
                                skip_group_check=True)

                def mm_n():
                    for i, m in enumerate((4, 5)):
                        for k in range(2):
                            nc.tensor.matmul(png[:, ds(nbase + i * NB, NB)],
                                             wh[:, ts(m * 2 + k, 128)],
                                             hist[:, ds(prev * W + k * NB, NB)],
                                             start=(k == 0), stop=(k == 1))

                if sig_split and not no_mm and not mm_only:
                    mm_rz((0, 1))
                    mm_n()
                    mm_rz((2, 3))
                    psv = t_ps[:].rearrange("p (m s b) -> p m s b", s=C, b=NB)
                    yield
                    r_t = work.tile([128, 2 * NB], F32, tag=f"rs{layer}")
                    nc.scalar.activation(
                        r_t[:].rearrange("p (m b) -> p m b", b=NB),
                        psv[:, 0:2, t % C], AF.Sigmoid)
                    yield
                    tt = work.tile([128, 2 * NB], F32, tag=f"tt{layer}")
                    nc.vector.tensor_mul(tt[:], r_t[:], png[:, ds(nbase, W)])
                    yield
                    z_t = work.tile([128, 2 * NB], F32, tag=f"zs{layer}")
                    nc.scalar.activation(
                        z_t[:].rearrange("p (m b) -> p m b", b=NB),
                        psv[:, 2:4, t % C], AF.Sigmoid)
                    yield
                    t2 = work.tile([128, 2 * NB], F32, tag=f"t2{layer}")
                    nc.vector.tensor_add(t2[:], tt[:],
                                         xn[:, ds(cur * 2 * NB, 2 * NB)])
                    yield
                    zh = work.tile([128, 2 * NB], F32, tag=f"zh{layer}")
                    nc.vector.tensor_mul(zh[:], z_t[:],
                                         hist[:, ds(prev * W, W)])
                    yield
                    nn = work.tile([128, 2 * NB], F32, tag=f"nn{layer}")
                    nc.scalar.activation(nn[:], t2[:], AF.Tanh)
                    yield
                    tmp = work.tile([128, 2 * NB], F32, tag=f"tm{layer}")
                    nc.vector.scalar_tensor_tensor(tmp[:], z_t[:], 1.0, nn[:],
                                                   ALU.subtract, ALU.mult)
                    yield
                    nc.vector.tensor_sub(hist[:, ds(cur * W, W)], zh[:],
                                         tmp[:])
                    return

                if not no_mm:
                    mm_rz((0, 1, 2, 3))
                    mm_n()
                else:
                    nc.vector.memset(png[:, ds(nbase, W)], 0.125)
                if mm_only:
                    return
                yield
                if dbg and layer == 0 and t == 0:
                    psrc = t_ps[:].rearrange("p (m s b) -> p m s b", s=C, b=NB)
                    dcp = work.tile([128, 4 * NB], F32, tag="dbgc2")
                    nc.vector.tensor_copy(
                        dcp[:].rearrange("p (m b) -> p m b", b=NB), psrc[:, :, 0])
                    nc.sync.dma_start(dbg_handles["dbg_ps"][:, 0:32], dcp[:])
                rz = work.tile([128, 4 * NB], F32, tag=f"rz{layer}")
                rzsrc = t_ps[:].rearrange("p (m s b) -> p m s b", s=C, b=NB)
                rzsrc = rzsrc[:, :, t % C]
                nc.scalar.activation(rz[:].rearrange("p (m b) -> p m b", b=NB),
                                     rzsrc, AF.Sigmoid)
                yield
                # n-path: t2 = r*hn + xn
                tt = work.tile([128, 2 * NB], F32, tag=f"tt{layer}")
                nc.vector.tensor_mul(tt[:], rz[:, 0:2 * NB],
                                     png[:, ds(nbase, W)])
                yield
                t2 = work.tile([128, 2 * NB], F32, tag=f"t2{layer}")
                nc.vector.tensor_add(t2[:], tt[:],
                                     xn[:, ds(cur * 2 * NB, 2 * NB)])
                yield
                # off-chain: zh = z * h_prev (GPSIMD, SBUF-only)
                zh = work.tile([128, 2 * NB], F32, tag=f"zh{layer}")
                _zh_eng = nc.gpsimd if os.environ.get("KZHGP", "0") == "1" else nc.vector
                _zh_eng.tensor_mul(zh[:], rz[:, ds(2 * NB, 2 * NB)],
                                   hist[:, ds(prev * W, W)])
                yield
                nn = work.tile([128, 2 * NB], F32, tag=f"nn{layer}")
                nc.scalar.activation(nn[:], t2[:], AF.Tanh)
                yield
                # h' = zh - (z-1)*n  == z*h + (1-z)*n
                tmp = work.tile([128, 2 * NB], F32, tag=f"tm{layer}")
                nc.vector.scalar_tensor_tensor(tmp[:], rz[:, ds(2 * NB, 2 * NB)],
                                               1.0, nn[:],
                                               ALU.subtract, ALU.mult)
                yield
                nc.vector.tensor_sub(hist[:, ds(cur * W, W)], zh[:], tmp[:])

            def emit_round(r):
                gens = []
                if r < T:
                    gens.append(step_gen(0, r))
                if r >= D:
                    gens.append(step_gen(1, r - D))
                while gens:
                    gens = [g for g in gens if next(g, "done") != "done"]

            no_aux = os.environ.get("KNOAUX", "0") == "1"

            def emit_body():
                # zero initial h slots (slot S-1 == slot(-1))
                nc.vector.memset(h1h[:, ds((S - 1) * W, W)], 0.0)
                nc.vector.memset(h2h[:, ds((S - 1) * W, W)], 0.0)
                emit_xg0_chunk(0)
                emit_xg0_chunk(1)
                while aux_q:
                    aux_q.pop(0)()
                n_rounds = T + D
                AUXK = int(os.environ.get("KAUXK", "3"))
                for r in range(n_rounds):
                    emit_round(r)
                    if r < T and (r + 1) % C == 0:
                        c = (r + 1) // C - 1  # layer-0 chunk just finished
                        emit_xg1_chunk(c)
                        if c + 2 < T // C:
                            emit_xg0_chunk(c + 2)
                    if r >= D and (r - D + 1) % CR == 0:
                        emit_regressor_chunk((r - D + 1) // CR - 1)
                    nd = 0
                    while aux_q and nd < AUXK:
                        aux_q.pop(0)()
                        nd += 1
                while aux_q:
                    aux_q.pop(0)()
                if dbg:
                    for nm, tl in [("dbg_h1", h1h), ("dbg_h2", h2h),
                                   ("dbg_x0", xn0), ("dbg_x1", xn1)]:
                        cp = work.tile([128, 64 * 16], F32, tag="dbgc")
                        nc.vector.tensor_copy(cp[:], tl[:])
                        nc.sync.dma_start(dbg_handles[nm][:, :], cp[:])

            if repeat == 1:
                emit_body()
            else:
                with tc.For_i(0, repeat, 1):
                    emit_body()

    nc.compile()
    return nc


_CACHE = {}


def _get_program(dt=BF16, repeat=1):
    key = (str(dt), repeat)
    if key not in _CACHE:
        _CACHE[key] = build_program(dt, repeat)
    return _CACHE[key]


def make_in_maps(inputs, np_dt=None):
    """Host-side prep: slice batch, transpose, pack biases."""
    if np_dt is None:
        np_dt = NP_DT
    x = np.asarray(inputs["x"], np.float32)
    Wih0 = np.asarray(inputs["Wih0"], np.float32)
    Whh0 = np.asarray(inputs["Whh0"], np.float32)
    bih0 = np.asarray(inputs["bih0"], np.float32)
    bhh0 = np.asarray(inputs["bhh0"], np.float32)
    Wih1 = np.asarray(inputs["Wih1"], np.float32)
    Whh1 = np.asarray(inputs["Whh1"], np.float32)
    bih1 = np.asarray(inputs["bih1"], np.float32)
    bhh1 = np.asarray(inputs["bhh1"], np.float32)
    W1 = np.asarray(inputs["W1"], np.float32)
    b1 = np.asarray(inputs["b1"], np.float32)
    W2 = np.asarray(inputs["W2"], np.float32)
    b2 = np.asarray(inputs["b2"], np.float32)

    assert not np.any(bhh0[2 * H:]) and not np.any(bhh1[2 * H:]), \
        "nonzero bhh n-gate bias not supported by this build"
    assert not np.any(bih1) and not np.any(bhh1[:2 * H]), \
        "nonzero layer-1 input bias not supported by this build"

    bias0 = np.concatenate([bih0[:2 * H] + bhh0[:2 * H], bih0[2 * H:]])
    x_dt = NP_XDT if NP_XDT is not None else np_dt
    wih0T = np.vstack([Wih0.T, bias0[None, :]]).astype(x_dt)  # [17, 768]

    shared = {
        "wh0T": Whh0.T.copy().astype(np_dt),
        "wih0T": wih0T,
        "wh1T": Whh1.T.copy().astype(np_dt),
        "wih1T": Wih1.T.copy().astype(np_dt),
        "w1T": W1.T.copy().astype(np.float32),
        "b1c": b1.reshape(2, 128).T.copy().astype(np.float32),
        "w2c": W2[0].reshape(2, 128).T.copy().astype(np.float32),
        "b2c": b2.reshape(1, 1).astype(np.float32),
    }
    in_maps = []
    for c in range(N_CORES):
        xc = x[c * Bc:(c + 1) * Bc]  # [8, T, 16]
        xTc = xc.transpose(2, 1, 0).reshape(I_DIM, T * Bc)  # [16, T*8]
        xTc = np.vstack([xTc, np.ones((1, T * Bc), np.float32)]).astype(x_dt)
        m = dict(shared)
        m["xT"] = xTc
        in_maps.append(m)
    return in_maps


def assemble_output(results):
    outs = []
    for c in range(N_CORES):
        r = np.asarray(results[c]["out"], np.float32)  # [T//CR, CR*Bc]
        r = r.reshape(T // CR, CR, Bc).transpose(2, 0, 1).reshape(Bc, T)
        outs.append(r)
    return np.concatenate(outs, axis=0)[:, :, None]  # [64, 2048, 1]


import ml_dtypes

X_F32 = os.environ.get("KXF32", "0") == "1"

if os.environ.get("KWF32", "0") == "1":
    DT_COMPUTE = F32
    NP_DT = np.float32
else:
    DT_COMPUTE = BF16
    NP_DT = ml_dtypes.bfloat16
NP_XDT = np.float32 if X_F32 else None  # None -> use NP_DT


def kernel(**inputs):
    nc = _get_program(DT_COMPUTE, 1)
    in_maps = make_in_maps(inputs, NP_DT)
    res = run_bass_kernel_spmd(nc, in_maps, core_ids=list(range(N_CORES)))
    return assemble_output(res.results)



# revision 3
# speedup vs baseline: 2.7054x; 2.7054x over previous
"""Trainium2 Bass kernel for a 2-layer GRU (B=64, T=2048, I=16, H=256) + MLP regressor.

Strategy (v3):
  - Data parallel: batch 64 sharded as 8 sequences per NeuronCore.
  - Each core runs BOTH GRU layers, software-pipelined with a D-step skew so the
    two layers' serial gate chains hide under each other's TensorE work.
  - All recurrent matmul operands bf16 (stationary weights get FWL: 2x faster
    LDWEIGHTS; carried h state is bf16). Regressor path kept fp32.
  - Layout: gates-on-partitions. r,z-gate input projections are precomputed
    into PSUM chunk accumulators; the per-step recurrent matmuls accumulate
    ONTO them (start=False), so sigmoid reads PSUM directly - no DVE add.
  - Gate chain per step (short critical path):
      rz = sigmoid(psum_rz); tt = r*hn; t2 = tt + xn; n = tanh(t2);
      tmp = (z-1)*n; h = zh - tmp.   zh = z*h_prev runs on GPSIMD off-chain.
  - n-gate input projections go to a small SBUF ring (one strided ACT copy
    per chunk); regressor fused every CR steps (fp32, h2 cast on GPSIMD).
"""

import os
import sys

import numpy as np

if "/opt/trn_rl_repo" not in sys.path:
    sys.path.insert(0, "/opt/trn_rl_repo")

import concourse.bacc as bacc
import concourse.mybir as mybir
import concourse.tile as tile
from concourse.bass import ds, ts
from concourse.bass_utils import run_bass_kernel_spmd

# Problem constants (hardcoded per harness contract)
B_TOTAL = 64
N_CORES = 8
Bc = B_TOTAL // N_CORES  # 8 sequences per core
T = 2048
I_DIM = 16
H = 256
G = 3 * H  # 768 gate rows
C = 16  # gate-chunk size (PSUM rz accumulators, xn ring refill)
CR = 32  # regressor chunk size
S = 64  # ring size in steps
D = 24  # layer-1 skew (steps)

F32 = mybir.dt.float32
BF16 = mybir.dt.bfloat16

AF = mybir.ActivationFunctionType
ALU = mybir.AluOpType


def build_program(dt_compute=BF16, repeat=1):
    """Build + compile the SPMD program (identical on all 8 cores)."""
    DT = dt_compute
    XDT = F32 if X_F32 else DT
    nc = bacc.Bacc("TRN2", target_bir_lowering=False, debug=False,
                   num_devices=N_CORES)

    # ---- DRAM I/O ----
    xT_h = nc.dram_tensor("xT", [I_DIM + 1, T * Bc], XDT, kind="ExternalInput")
    wh0_h = nc.dram_tensor("wh0T", [H, G], DT, kind="ExternalInput")
    wih0_h = nc.dram_tensor("wih0T", [I_DIM + 1, G], XDT, kind="ExternalInput")
    wh1_h = nc.dram_tensor("wh1T", [H, G], DT, kind="ExternalInput")
    wih1_h = nc.dram_tensor("wih1T", [H, G], DT, kind="ExternalInput")
    w1_h = nc.dram_tensor("w1T", [H, H], F32, kind="ExternalInput")
    b1_h = nc.dram_tensor("b1c", [128, 2], F32, kind="ExternalInput")
    w2_h = nc.dram_tensor("w2c", [128, 2], F32, kind="ExternalInput")
    b2_h = nc.dram_tensor("b2c", [1, 1], F32, kind="ExternalInput")
    out_h = nc.dram_tensor("out", [T // CR, CR * Bc], F32, kind="ExternalOutput")
    dbg = os.environ.get("KDBG", "0") == "1"
    dbg_handles = {}
    if dbg:
        for nm in ("dbg_h1", "dbg_h2", "dbg_x0", "dbg_x1", "dbg_ps"):
            dbg_handles[nm] = nc.dram_tensor(nm, [128, 64 * 16], F32,
                                             kind="ExternalOutput")

    NB = Bc  # batch per core
    W = NB * 2  # 16: one h-state slot width (2 k-chunks x 8)
    CB = C * NB  # 128: columns per (m-chunk, gate-chunk)

    with tile.TileContext(nc) as tc:
        with (
            tc.tile_pool(name="cst", bufs=1) as cst,
            tc.tile_pool(name="work", bufs=3) as work,
            tc.tile_pool(name="prz", bufs=2, space="PSUM") as prz,
            tc.tile_pool(name="pgg", bufs=1, space="PSUM") as pgg,
            tc.tile_pool(name="pbig", bufs=2, space="PSUM") as pbig,
        ):
            # ---- persistent SBUF ----
            xT = cst.tile([I_DIM + 1, T * NB], XDT, tag="xT")
            wh0 = cst.tile([128, 12 * 128], DT, tag="wh0")
            wh1 = cst.tile([128, 12 * 128], DT, tag="wh1")
            wih0 = cst.tile([I_DIM + 1, G], XDT, tag="wih0")
            wih1 = cst.tile([128, 2 * G], DT, tag="wih1")
            w1 = cst.tile([128, 4 * 128], F32, tag="w1")
            w2 = cst.tile([128, 2], F32, tag="w2")
            b1 = cst.tile([128, 2], F32, tag="b1")
            b2 = cst.tile([1, 1], F32, tag="b2")
            xn0 = cst.tile([128, S * 2 * NB], F32, tag="xn0")  # n-gate ring
            xn1 = cst.tile([128, S * 2 * NB], F32, tag="xn1")
            h1h = cst.tile([128, S * W], DT, tag="h1h")  # h1.T history ring
            h2h = cst.tile([128, S * W], DT, tag="h2h")
            # persistent per-step n-gate psum: (layer, t%2) -> 16-col region
            png = pgg.tile([128, 4 * W], F32, tag="gg")

            # ---- load constants ----
            nc.sync.dma_start(xT[:], xT_h[:, :])
            for m in range(6):
                for k in range(2):
                    i = m * 2 + k
                    nc.sync.dma_start(wh0[:, ts(i, 128)],
                                      wh0_h[ds(k * 128, 128), ds(m * 128, 128)])
                    nc.sync.dma_start(wh1[:, ts(i, 128)],
                                      wh1_h[ds(k * 128, 128), ds(m * 128, 128)])
            nc.sync.dma_start(wih0[:], wih0_h[:, :])
            for k in range(2):
                nc.sync.dma_start(wih1[:, ts(k, G)], wih1_h[ds(k * 128, 128), :])
            for mm in range(2):
                for k in range(2):
                    nc.sync.dma_start(w1[:, ts(mm * 2 + k, 128)],
                                      w1_h[ds(k * 128, 128), ds(mm * 128, 128)])
            nc.sync.dma_start(w2[:], w2_h[:, :])
            nc.sync.dma_start(b1[:], b1_h[:, :])
            nc.sync.dma_start(b2[:], b2_h[:, :])

            # live rz-psum chunk accumulators, per layer, by chunk parity
            psA = {0: [None, None], 1: [None, None]}

            aux_q = []
            copv = os.environ.get("KCOPV", "0") == "1"

            def emit_xg0_chunk(c):
                """Layer-0 input projections for steps [c*C, (c+1)*C)."""
                xchunk = xT[:, ds(c * CB, CB)]
                state = {}

                def u_rz(m0):
                    def f():
                        if m0 == 0:
                            state["t_ps"] = prz.tile([128, 4 * CB], F32,
                                                     tag="rz0", name="tps0")
                            psA[0][c % 2] = state["t_ps"]
                        for m in (m0, m0 + 1):
                            nc.tensor.matmul(state["t_ps"][:, ds(m * CB, CB)],
                                             wih0[:, ts(m, 128)], xchunk,
                                             start=(m == 0), stop=(m == 3))
                    return f

                def u_n():
                    pn = pbig.tile([128, 2 * CB], F32, tag="big")
                    state["pn"] = pn
                    for i, m in enumerate((4, 5)):
                        nc.tensor.matmul(pn[:, ds(i * CB, CB)],
                                         wih0[:, ts(m, 128)], xchunk,
                                         start=True, stop=True)

                def u_copy():
                    pn = state["pn"]
                    dst = xn0[:, ds((c * C % S) * 2 * NB, C * 2 * NB)]
                    dst = dst.rearrange("p (s m b) -> p s m b", m=2, b=NB)
                    srcv = pn[:].rearrange("p (m s b) -> p s m b", m=2, b=NB)
                    if copv:
                        nc.vector.tensor_copy(dst, srcv)
                    else:
                        nc.scalar.activation(dst, srcv, AF.Copy)

                aux_q.extend([u_rz(0), u_rz(2), u_n, u_copy])

            def emit_xg1_chunk(c):
                """Layer-1 input projections for steps [c*C, (c+1)*C)."""
                base_step = (c * C) % S
                seg = h1h[:, ds(base_step * W, C * W)]
                seg = seg.rearrange("p (s k b) -> p k s b", k=2, b=NB)
                state = {}

                def u_rz(m0):
                    def f():
                        if m0 == 0:
                            state["t_ps"] = prz.tile([128, 4 * CB], F32,
                                                     tag="rz1", name="tps1")
                            psA[1][c % 2] = state["t_ps"]
                        for m in (m0, m0 + 1):
                            for k in range(2):
                                nc.tensor.matmul(
                                    state["t_ps"][:, ds(m * CB, CB)],
                                    wih1[:, ds(k * G + m * 128, 128)],
                                    seg[:, k],
                                    start=(m == 0 and k == 0),
                                    stop=(m == 3 and k == 1))
                    return f

                def u_n(i, m):
                    def f():
                        if i == 0:
                            state["pn"] = pbig.tile([128, 2 * CB], F32,
                                                    tag="big", name="pn1")
                        for k in range(2):
                            nc.tensor.matmul(state["pn"][:, ds(i * CB, CB)],
                                             wih1[:, ds(k * G + m * 128, 128)],
                                             seg[:, k],
                                             start=(k == 0), stop=(k == 1))
                    return f

                def u_copy():
                    pn = state["pn"]
                    dst = xn1[:, ds(base_step * 2 * NB, C * 2 * NB)]
                    dst = dst.rearrange("p (s m b) -> p s m b", m=2, b=NB)
                    srcv = pn[:].rearrange("p (m s b) -> p s m b", m=2, b=NB)
                    if copv:
                        nc.vector.tensor_copy(dst, srcv)
                    else:
                        nc.scalar.activation(dst, srcv, AF.Copy)

                aux_q.extend([u_rz(0), u_rz(2), u_n(0, 4), u_n(1, 5), u_copy])

            def bias_relu(out, ps, bcol):
                if copv:
                    nc.vector.tensor_scalar(out, ps, bcol, 0.0,
                                            ALU.add, ALU.max)
                else:
                    nc.scalar.activation(out, ps, AF.Relu, bias=bcol)

            def emit_regressor_chunk(rc):
                """relu(h2@W1.T+b1) @ W2.T + b2 -> relu -> out, for steps
                [rc*CR, (rc+1)*CR) of layer 1. fp32 path."""
                base_step = (rc * CR) % S
                state = {}

                def u_seg():
                    segf = work.tile([128, CR * W], F32, tag="segf",
                                     name="segf")
                    nc.gpsimd.tensor_copy(segf[:],
                                          h2h[:, ds(base_step * W, CR * W)])
                    state["seg"] = segf[:].rearrange("p (s k b) -> p k s b",
                                                     k=2, b=NB)
                    state["rT"] = work.tile([128, 2 * CR * NB], F32,
                                            tag="rT", name="rTt")

                def u_mm(mm, k):
                    def f():
                        if k == 0:
                            state[f"ps{mm}"] = pbig.tile([128, CR * NB], F32,
                                                         tag="big",
                                                         name="psr")
                        nc.tensor.matmul(state[f"ps{mm}"][:],
                                         w1[:, ts(mm * 2 + k, 128)],
                                         state["seg"][:, k],
                                         start=(k == 0), stop=(k == 1))
                        if k == 1:
                            bias_relu(state["rT"][:, ts(mm, CR * NB)],
                                      state[f"ps{mm}"][:], b1[:, ds(mm, 1)])
                    return f

                def u_out():
                    pot = pbig.tile([128, CR * NB], F32, tag="big",
                                    name="pot")
                    po = pot[0:1, :]
                    for k in range(2):
                        nc.tensor.matmul(po, w2[:, ds(k, 1)],
                                         state["rT"][:, ts(k, CR * NB)],
                                         start=(k == 0), stop=(k == 1))
                    oT = work.tile([1, CR * NB], F32, tag="oT", name="oT")
                    bias_relu(oT[:], po, b2[:, ds(0, 1)])
                    nc.sync.dma_start(out_h[ds(rc, 1), :], oT[:])

                aux_q.extend([u_seg, u_mm(0, 0), u_mm(0, 1), u_mm(1, 0),
                              u_mm(1, 1), u_out])

            mm_only = os.environ.get("KMMONLY", "0") == "1"
            no_mm = os.environ.get("KNOMM", "0") == "1"
            sig_split = os.environ.get("KSIGSPLIT", "0") == "1"

            def step_gen(layer, t):
                """Generator emitting one GRU step; yields between chain ops so
                the two layers' chains can be interleaved per-engine."""
                wh = wh0 if layer == 0 else wh1
                hist = h1h if layer == 0 else h2h
                xn = xn0 if layer == 0 else xn1
                t_ps = psA[layer][(t // C) % 2]
                tc_off = (t % C) * NB
                prev = (t - 1) % S
                cur = t % S
                nbase = (layer * 2 + t % 2) * W

                def mm_rz(ms):
                    for m in ms:
                        for k in range(2):
                            nc.tensor.matmul(
                                t_ps[:, ds(m * CB + tc_off, NB)],
                                wh[:, ts(m * 2 + k, 128)],
                                hist[:, ds(prev * W + k * NB, NB)],
                                start=False, stop=(k == 1),
                                skip_group_check=True)

                def mm_n():
                    for i, m in enumerate((4, 5)):
                        for k in range(2):
                            nc.tensor.matmul(png[:, ds(nbase + i * NB, NB)],
                                             wh[:, ts(m * 2 + k, 128)],
                                             hist[:, ds(prev * W + k * NB, NB)],
                                             start=(k == 0), stop=(k == 1))

                if sig_split and not no_mm and not mm_only:
                    mm_rz((0, 1))
                    mm_n()
                    mm_rz((2, 3))
                    psv = t_ps[:].rearrange("p (m s b) -> p m s b", s=C, b=NB)
                    yield
                    r_t = work.tile([128, 2 * NB], F32, tag=f"rs{layer}")
                    nc.scalar.activation(
                        r_t[:].rearrange("p (m b) -> p m b", b=NB),
                        psv[:, 0:2, t % C], AF.Sigmoid)
                    yield
                    tt = work.tile([128, 2 * NB], F32, tag=f"tt{layer}")
                    nc.vector.tensor_mul(tt[:], r_t[:], png[:, ds(nbase, W)])
                    yield
                    z_t = work.tile([128, 2 * NB], F32, tag=f"zs{layer}")
                    nc.scalar.activation(
                        z_t[:].rearrange("p (m b) -> p m b", b=NB),
                        psv[:, 2:4, t % C], AF.Sigmoid)
                    yield
                    t2 = work.tile([128, 2 * NB], F32, tag=f"t2{layer}")
                    nc.vector.tensor_add(t2[:], tt[:],
                                         xn[:, ds(cur * 2 * NB, 2 * NB)])
                    yield
                    zh = work.tile([128, 2 * NB], F32, tag=f"zh{layer}")
                    nc.vector.tensor_mul(zh[:], z_t[:],
                                         hist[:, ds(prev * W, W)])
                    yield
                    nn = work.tile([128, 2 * NB], F32, tag=f"nn{layer}")
                    nc.scalar.activation(nn[:], t2[:], AF.Tanh)
                    yield
                    tmp = work.tile([128, 2 * NB], F32, tag=f"tm{layer}")
                    nc.vector.scalar_tensor_tensor(tmp[:], z_t[:], 1.0, nn[:],
                                                   ALU.subtract, ALU.mult)
                    yield
                    nc.vector.tensor_sub(hist[:, ds(cur * W, W)], zh[:],
                                         tmp[:])
                    return

                if not no_mm:
                    mm_rz((0, 1, 2, 3))
                    mm_n()
                else:
                    nc.vector.memset(png[:, ds(nbase, W)], 0.125)
                if mm_only:
                    return
                yield
                if dbg and layer == 0 and t == 0:
                    psrc = t_ps[:].rearrange("p (m s b) -> p m s b", s=C, b=NB)
                    dcp = work.tile([128, 4 * NB], F32, tag="dbgc2")
                    nc.vector.tensor_copy(
                        dcp[:].rearrange("p (m b) -> p m b", b=NB), psrc[:, :, 0])
                    nc.sync.dma_start(dbg_handles["dbg_ps"][:, 0:32], dcp[:])
                rz = work.tile([128, 4 * NB], F32, tag=f"rz{layer}")
                rzsrc = t_ps[:].rearrange("p (m s b) -> p m s b", s=C, b=NB)
                rzsrc = rzsrc[:, :, t % C]
                nc.scalar.activation(rz[:].rearrange("p (m b) -> p m b", b=NB),
                                     rzsrc, AF.Sigmoid)
                yield
                # n-path: t2 = r*hn + xn
                tt = work.tile([128, 2 * NB], F32, tag=f"tt{layer}")
                nc.vector.tensor_mul(tt[:], rz[:, 0:2 * NB],
                                     png[:, ds(nbase, W)])
                yield
                t2 = work.tile([128, 2 * NB], F32, tag=f"t2{layer}")
                nc.vector.tensor_add(t2[:], tt[:],
                                     xn[:, ds(cur * 2 * NB, 2 * NB)])
                yield
                # off-chain: zh = z * h_prev (GPSIMD, SBUF-only)
                zh = work.tile([128, 2 * NB], F32, tag=f"zh{layer}")
                _zh_eng = nc.gpsimd if os.environ.get("KZHGP", "0") == "1" else nc.vector
                _zh_eng.tensor_mul(zh[:], rz[:, ds(2 * NB, 2 * NB)],
                                   hist[:, ds(prev * W, W)])
                yield
                nn = work.tile([128, 2 * NB], F32, tag=f"nn{layer}")
                nc.scalar.activation(nn[:], t2[:], AF.Tanh)
                yield
                # h' = zh - (z-1)*n  == z*h + (1-z)*n
                tmp = work.tile([128, 2 * NB], F32, tag=f"tm{layer}")
                nc.vector.scalar_tensor_tensor(tmp[:], rz[:, ds(2 * NB, 2 * NB)],
                                               1.0, nn[:],
                                               ALU.subtract, ALU.mult)
                yield
                nc.vector.tensor_sub(hist[:, ds(cur * W, W)], zh[:], tmp[:])

            sched = os.environ.get("KSCHED", "rr")
            carry = [None]  # L1 generator carried across rounds (half sched)

            def _adv(g, n):
                """Advance generator g by n yields; return False when done."""
                if g is None:
                    return False
                for _ in range(n):
                    if next(g, "done") == "done":
                        return False
                return True

            def emit_round(r):
                if sched == "rr":
                    gens = []
                    if r < T:
                        gens.append(step_gen(0, r))
                    if r >= D:
                        gens.append(step_gen(1, r - D))
                    while gens:
                        gens = [g for g in gens if next(g, "done") != "done"]
                elif sched == "seq":
                    if r < T:
                        g = step_gen(0, r)
                        while next(g, "done") != "done":
                            pass
                    if r >= D:
                        g = step_gen(1, r - D)
                        while next(g, "done") != "done":
                            pass
                elif sched == "half":
                    # L1's step chain is phase-shifted half a step vs L0:
                    # [L0 mm,sig,mul,add | L1(prev) tanh,stt,sub | L0 zh,
                    #  tanh,stt,sub | L1 mm,sig,mul,add,zh]
                    g0 = step_gen(0, r) if r < T else None
                    alive0 = _adv(g0, 4) if g0 else False  # mm,sig,mul,add
                    if carry[0] is not None:
                        while next(carry[0], "done") != "done":
                            pass
                        carry[0] = None
                    if alive0:
                        while next(g0, "done") != "done":
                            pass
                    if r >= D:
                        g1 = step_gen(1, r - D)
                        if _adv(g1, 5):  # mm,sig,mul,add,zh
                            carry[0] = g1
                else:
                    raise ValueError(f"bad KSCHED {sched}")

            no_aux = os.environ.get("KNOAUX", "0") == "1"

            def emit_body():
                # zero initial h slots (slot S-1 == slot(-1))
                nc.vector.memset(h1h[:, ds((S - 1) * W, W)], 0.0)
                nc.vector.memset(h2h[:, ds((S - 1) * W, W)], 0.0)
                emit_xg0_chunk(0)
                emit_xg0_chunk(1)
                while aux_q:
                    aux_q.pop(0)()
                n_rounds = T + D
                AUXK = int(os.environ.get("KAUXK", "3"))
                for r in range(n_rounds):
                    emit_round(r)
                    if r < T and (r + 1) % C == 0:
                        c = (r + 1) // C - 1  # layer-0 chunk just finished
                        emit_xg1_chunk(c)
                        if c + 2 < T // C:
                            emit_xg0_chunk(c + 2)
                    if r >= D and (r - D + 1) % CR == 0:
                        emit_regressor_chunk((r - D + 1) // CR - 1)
                    nd = 0
                    while aux_q and nd < AUXK:
                        aux_q.pop(0)()
                        nd += 1
                if carry[0] is not None:
                    while next(carry[0], "done") != "done":
                        pass
                    carry[0] = None
                while aux_q:
                    aux_q.pop(0)()
                if dbg:
                    for nm, tl in [("dbg_h1", h1h), ("dbg_h2", h2h),
                                   ("dbg_x0", xn0), ("dbg_x1", xn1)]:
                        cp = work.tile([128, 64 * 16], F32, tag="dbgc")
                        nc.vector.tensor_copy(cp[:], tl[:])
                        nc.sync.dma_start(dbg_handles[nm][:, :], cp[:])

            if repeat == 1:
                emit_body()
            else:
                with tc.For_i(0, repeat, 1):
                    emit_body()

    nc.compile()
    return nc


_CACHE = {}


def _get_program(dt=BF16, repeat=1):
    key = (str(dt), repeat)
    if key not in _CACHE:
        _CACHE[key] = build_program(dt, repeat)
    return _CACHE[key]


def make_in_maps(inputs, np_dt=None):
    """Host-side prep: slice batch, transpose, pack biases."""
    if np_dt is None:
        np_dt = NP_DT
    x = np.asarray(inputs["x"], np.float32)
    Wih0 = np.asarray(inputs["Wih0"], np.float32)
    Whh0 = np.asarray(inputs["Whh0"], np.float32)
    bih0 = np.asarray(inputs["bih0"], np.float32)
    bhh0 = np.asarray(inputs["bhh0"], np.float32)
    Wih1 = np.asarray(inputs["Wih1"], np.float32)
    Whh1 = np.asarray(inputs["Whh1"], np.float32)
    bih1 = np.asarray(inputs["bih1"], np.float32)
    bhh1 = np.asarray(inputs["bhh1"], np.float32)
    W1 = np.asarray(inputs["W1"], np.float32)
    b1 = np.asarray(inputs["b1"], np.float32)
    W2 = np.asarray(inputs["W2"], np.float32)
    b2 = np.asarray(inputs["b2"], np.float32)

    assert not np.any(bhh0[2 * H:]) and not np.any(bhh1[2 * H:]), \
        "nonzero bhh n-gate bias not supported by this build"
    assert not np.any(bih1) and not np.any(bhh1[:2 * H]), \
        "nonzero layer-1 input bias not supported by this build"

    bias0 = np.concatenate([bih0[:2 * H] + bhh0[:2 * H], bih0[2 * H:]])
    x_dt = NP_XDT if NP_XDT is not None else np_dt
    wih0T = np.vstack([Wih0.T, bias0[None, :]]).astype(x_dt)  # [17, 768]

    shared = {
        "wh0T": Whh0.T.copy().astype(np_dt),
        "wih0T": wih0T,
        "wh1T": Whh1.T.copy().astype(np_dt),
        "wih1T": Wih1.T.copy().astype(np_dt),
        "w1T": W1.T.copy().astype(np.float32),
        "b1c": b1.reshape(2, 128).T.copy().astype(np.float32),
        "w2c": W2[0].reshape(2, 128).T.copy().astype(np.float32),
        "b2c": b2.reshape(1, 1).astype(np.float32),
    }
    in_maps = []
    for c in range(N_CORES):
        xc = x[c * Bc:(c + 1) * Bc]  # [8, T, 16]
        xTc = xc.transpose(2, 1, 0).reshape(I_DIM, T * Bc)  # [16, T*8]
        xTc = np.vstack([xTc, np.ones((1, T * Bc), np.float32)]).astype(x_dt)
        m = dict(shared)
        m["xT"] = xTc
        in_maps.append(m)
    return in_maps


def assemble_output(results):
    outs = []
    for c in range(N_CORES):
        r = np.asarray(results[c]["out"], np.float32)  # [T//CR, CR*Bc]
        r = r.reshape(T // CR, CR, Bc).transpose(2, 0, 1).reshape(Bc, T)
        outs.append(r)
    return np.concatenate(outs, axis=0)[:, :, None]  # [64, 2048, 1]


import ml_dtypes

X_F32 = os.environ.get("KXF32", "0") == "1"

if os.environ.get("KWF32", "0") == "1":
    DT_COMPUTE = F32
    NP_DT = np.float32
else:
    DT_COMPUTE = BF16
    NP_DT = ml_dtypes.bfloat16
NP_XDT = np.float32 if X_F32 else None  # None -> use NP_DT


def kernel(**inputs):
    nc = _get_program(DT_COMPUTE, 1)
    in_maps = make_in_maps(inputs, NP_DT)
    res = run_bass_kernel_spmd(nc, in_maps, core_ids=list(range(N_CORES)))
    return assemble_output(res.results)

